# revision 1
# baseline (speedup 1.0000x reference)
"""Bass/Trainium2 kernel for nn_Bilinear (out[b,n,i] = enc[b,n,i,:] @ W @ hidden[b,:] + bias).

Sharding: data-parallel over B. 8 cores, one batch element each.
Per core:
  stage 1 (TensorE): v[j] = sum_k W[j,k] * h[k].  Host feeds Wt = W.T so the
    contraction dim k sits on SBUF partitions; Wt streams in as 8 chunked
    0.5 MiB DMAs (h/bias first, so matmuls only wait on their Wt chunk) and
    16 small matmuls pipeline behind them, accumulating v into PSUM.  v is
    partition-broadcast on the PE (ones[1,128].T @ v[1,512] -> [128,512])
    so no DMA sits on the v critical path.
  stage 2 (VectorE + ScalarE): stream enc rows as [128, 4, 1024] tiles
    (2 MiB DMAs); per 128-row block, 1-in-4 blocks use the fused custom-DVE
    TENSOR_TENSOR_REDUCE and the rest use DVE-mul + ScalarE accumulate-Copy,
    balancing both engines below the DMA rate so the kernel stays
    DMA-paced end to end.  The last chunks are tapered (1 MiB) to shorten
    the compute trail after the stream ends.  Bias is added once at the end.
Output is written per-core as out[b].T ([128 i, 64 n]); host transposes back.
"""

import numpy as np

B, N, I, H = 8, 64, 128, 1024
P = 128
NI = N * I  # 8192 rows per core
N_CORES = 8

_NC_CACHE = {}
LAST_RESULTS = None


def _build(ni_rows=NI, ebufs=8):
    import concourse.bacc as bacc
    import concourse.mybir as mybir
    import concourse.tile as tile
    from concourse import dve_ops

    f32 = mybir.dt.float32
    KB = H // P  # k blocks for stage 1
    n_blocks = ni_rows // P
    # chunk schedule in 128-row blocks: 2 MiB (4 blocks) for the bulk,
    # 1 MiB (2 blocks) for the last few to shorten the trailing compute
    tail_blocks = 8 if n_blocks > 8 else 0
    bulk = n_blocks - tail_blocks
    chunks = [4] * (bulk // 4) + [2] * (tail_blocks // 2)
    assert sum(chunks) == n_blocks

    nc = bacc.Bacc(
        "TRN2",
        target_bir_lowering=False,
        debug=False,
        num_devices=N_CORES,
    )
    enc = nc.declare_dram_parameter("enc", [ni_rows, H], f32, isOutput=False)
    hh = nc.declare_dram_parameter("h", [P, KB], f32, isOutput=False)
    wt = nc.declare_dram_parameter("wt", [H, H], f32, isOutput=False)
    bb = nc.declare_dram_parameter("bias", [1, 1], f32, isOutput=False)
    out = nc.declare_dram_parameter("out_t", [P, n_blocks], f32, isOutput=True)

    with tile.TileContext(nc) as tc:
        with (
            tc.tile_pool(name="const", bufs=1) as const,
            tc.tile_pool(name="epool", bufs=ebufs) as epool,
            tc.tile_pool(name="ppool", bufs=3) as ppool,
            tc.tile_pool(name="vpsum", bufs=1, space="PSUM") as vpsum,
        ):
            # ---- stage 1: v[j] = sum_k Wt[k,j] h[k] ----
            h_col = const.tile([P, KB], f32)
            nc.sync.dma_start(out=h_col[:], in_=hh[:, :])
            bias_col = const.tile([P, 1], f32)
            nc.sync.dma_start(out=bias_col[:], in_=bb[:, :].to_broadcast((P, 1)))
            wt_sbs = []
            for kb in range(KB):
                wt_kb = const.tile([P, H], f32, name=f"wt{kb}", tag=f"wt{kb}")
                nc.sync.dma_start(out=wt_kb[:], in_=wt[kb * P : (kb + 1) * P, :])
                wt_sbs.append(wt_kb)
            ones = const.tile([1, P], f32)
            nc.vector.memset(ones[:], 1.0)

            v_flat = const.tile([1, H], f32)
            vps = [
                vpsum.tile([1, 512], f32, name=f"vp{jc}", tag=f"vp{jc}")
                for jc in range(H // 512)
            ]
            for kb in range(KB):
                for jc in range(H // 512):
                    nc.tensor.matmul(
                        vps[jc][:],
                        h_col[:, kb : kb + 1],
                        wt_sbs[kb][:, jc * 512 : (jc + 1) * 512],
                        start=(kb == 0),
                        stop=(kb == KB - 1),
                    )
            for jc in range(H // 512):
                nc.scalar.activation(
                    v_flat[:, jc * 512 : (jc + 1) * 512],
                    vps[jc][:],
                    mybir.ActivationFunctionType.Copy,
                )
            # partition-broadcast v on the PE: ones[1,P].T @ v[1,512] -> [P,512]
            v_rep = const.tile([P, H], f32)
            for jc in range(H // 512):
                bc = vpsum.tile([P, 512], f32, name=f"bc{jc}", tag=f"bc{jc}")
                nc.tensor.matmul(
                    bc[:],
                    ones[:],
                    v_flat[:, jc * 512 : (jc + 1) * 512],
                    start=True,
                    stop=True,
                )
                nc.scalar.activation(
                    v_rep[:, jc * 512 : (jc + 1) * 512],
                    bc[:],
                    mybir.ActivationFunctionType.Copy,
                )

            # ---- stage 2: out[col*128+p] = sum_j enc[row, j] * v[j] ----
            # Per 4 blocks, 1 uses the fused all-DVE TTR and 3 use DVE-mul +
            # ScalarE accumulate-Copy, balancing the two engines (~5 us per
            # 2 MiB chunk each) under the ~5.5 us/chunk DMA.
            out_sb = const.tile([P, n_blocks], f32)
            dummy = const.tile([P, 1], f32)
            enc_b = enc[:, :].rearrange("(blk p) j -> blk p j", p=P)
            col = 0
            for ci, C in enumerate(chunks):
                e_tile = epool.tile([P, 4, H], f32, name=f"e{ci}", tag="e")
                nc.sync.dma_start(
                    out=e_tile[:, :C],
                    in_=enc_b[col : col + C].rearrange("blk p j -> p blk j"),
                )
                for c in range(C):
                    if col % 4 == 0:
                        nc.vector._custom_dve(
                            dve_ops.TENSOR_TENSOR_REDUCE,
                            out=dummy[:].broadcast_to((P, H)),
                            in0=e_tile[:, c],
                            in1=v_rep[:],
                            s0=0.0,
                            s1=1.0,
                            accum_out=out_sb[:, col : col + 1],
                        )
                    else:
                        prod = ppool.tile([P, H], f32)
                        nc.vector.tensor_mul(prod[:], e_tile[:, c], v_rep[:])
                        nc.scalar.activation(
                            prod[:],
                            prod[:],
                            mybir.ActivationFunctionType.Copy,
                            accum_out=out_sb[:, col : col + 1],
                        )
                    col += 1
            # bias + writeback: head columns overlap the last tail blocks'
            # compute (the stream is already drained by then); only the
            # final 4 columns stay serial after the last accumulate
            head = max(n_blocks - 4, 0)
            if head:
                nc.vector.tensor_scalar_add(
                    out_sb[:, :head], out_sb[:, :head], bias_col[:]
                )
                nc.sync.dma_start(out=out[:, :head], in_=out_sb[:, :head])
            nc.vector.tensor_scalar_add(
                out_sb[:, head:], out_sb[:, head:], bias_col[:]
            )
            nc.sync.dma_start(out=out[:, head:], in_=out_sb[:, head:])
    nc.compile()
    return nc


def _get_nc():
    if "nc" not in _NC_CACHE:
        _NC_CACHE["nc"] = _build()
    return _NC_CACHE["nc"]


def kernel(hidden=None, encoder_hiddens=None, input_lengths=None, W=None, b=None):
    global LAST_RESULTS
    from concourse.bass_utils import run_bass_kernel_spmd

    hidden = np.asarray(hidden, dtype=np.float32)
    enc = np.asarray(encoder_hiddens, dtype=np.float32)
    W_ = np.asarray(W, dtype=np.float32)
    b_ = np.asarray(b, dtype=np.float32).reshape(1, 1)
    wt = np.ascontiguousarray(W_.T)

    nc = _get_nc()
    KB = H // P
    in_maps = []
    for core in range(N_CORES):
        in_maps.append(
            {
                "enc": np.ascontiguousarray(enc[core].reshape(NI, H)),
                "h": np.ascontiguousarray(hidden[core].reshape(KB, P).T),
                "wt": wt,
                "bias": b_,
            }
        )
    res = run_bass_kernel_spmd(nc, in_maps, core_ids=list(range(N_CORES)))
    LAST_RESULTS = res
    out = np.stack([res.results[i]["out_t"].T for i in range(N_CORES)])
    return np.ascontiguousarray(out.astype(np.float32))



# revision 6
# speedup vs baseline: 1.7460x; 1.7460x over previous
"""Bass/Trainium2 kernel for nn_Bilinear (out[b,n,i] = enc[b,n,i,:] @ W @ hidden[b,:] + bias).

Sharding: data-parallel over B. 8 cores, one batch element each.

The kernel is DMA-bound (enc is 32 MiB/core at f32), so everything streams as
bf16 (harness gate is rel_err < 2e-2; measured bf16 error ~3e-3), halving HBM
traffic to ~18 MiB/core, and ALL math runs on the TensorE so Vector/Scalar
never pace the stream:

  host:    enc[b] is pre-transposed to enc_t [H=1024, N*I=8192] bf16 so the
           contraction dim H sits on SBUF partitions; W is fed as W.T bf16.
  stage 1: v[j] = sum_k W[j,k] h[k] on the PE: 64 matmuls with W.T chunks
           [128k, 128j] stationary and h chunks [128k, 1] moving, accumulating
           v as columns v_psum[128, 8] (v already partition-major for stage 2).
  stage 2: out[r] = sum_h enc_t[h, r] v[h]: per 128-h slab, 16 matmuls with
           v_col[:, hc] ([128, 1]) stationary and enc_t slab cols [128, 512]
           moving. The 16 row-groups accumulate into 4 PSUM banks x partitions
           {0, 32, 64, 96} (tile_position col-groups), so the whole 8192-row
           output lives in one [128, 2048] PSUM tile.
  tail:    one VectorE tensor_scalar_add applies bias while copying PSUM->SBUF;
           one 32 KiB DMA writes out[4, 2048]; host reshapes to [64, 128].

enc_t streams as 16 x 1 MiB DMAs (8 KiB/partition runs) into 8 resident SBUF
slabs (the full 16 MiB shard fits in SBUF); 1 MiB granularity keeps PE idle
gaps under the ~3.4 us HAM re-throttle window.
"""

import numpy as np
import ml_dtypes

B, N, I, H = 8, 64, 128, 1024
P = 128
NI = N * I  # 8192 rows per core
HC = H // P  # 8 h-chunks
N_CORES = 8
BF = ml_dtypes.bfloat16

_NC_CACHE = {}
LAST_RESULTS = None


def _build():
    import concourse.bacc as bacc
    import concourse.mybir as mybir
    import concourse.tile as tile

    f32 = mybir.dt.float32
    bf16 = mybir.dt.bfloat16

    nc = bacc.Bacc(
        "TRN2",
        target_bir_lowering=False,
        debug=False,
        num_devices=N_CORES,
    )
    enc_t = nc.declare_dram_parameter("enc_t", [H, NI], bf16, isOutput=False)
    wt = nc.declare_dram_parameter("wt", [H, H], bf16, isOutput=False)
    hh = nc.declare_dram_parameter("h", [P, HC], bf16, isOutput=False)
    bb = nc.declare_dram_parameter("bias", [1, 1], f32, isOutput=False)
    out = nc.declare_dram_parameter("out", [4, 4 * 512], f32, isOutput=True)

    with tile.TileContext(nc) as tc:
        with (
            tc.tile_pool(name="const", bufs=1) as const,
            tc.tile_pool(name="psum", bufs=1, space="PSUM") as psp,
        ):
            # ---- small loads ----
            h_col = const.tile([P, HC], bf16)
            nc.sync.dma_start(out=h_col[:], in_=hh[:, :])
            bias_col = const.tile([P, 1], f32)
            nc.sync.dma_start(out=bias_col[:], in_=bb[:, :].to_broadcast((P, 1)))

            # ---- W.T as two 1 MiB DMAs, kc-major in the free dim ----
            w_sb = []
            for wi in range(2):
                wtile = const.tile([P, 4, H], bf16, name=f"w{wi}")
                nc.sync.dma_start(
                    out=wtile[:],
                    in_=wt[wi * 512 : (wi + 1) * 512, :].rearrange(
                        "(kc p) j -> p kc j", p=P
                    ),
                )
                w_sb.append(wtile)

            # ---- enc_t: 8 resident slabs, 2 x 1 MiB DMAs each ----
            e_sb = [const.tile([P, NI], bf16, name=f"e{hc}") for hc in range(HC)]
            for hc in range(HC):
                for half in range(2):
                    j0 = half * (NI // 2)
                    nc.sync.dma_start(
                        out=e_sb[hc][:, j0 : j0 + NI // 2],
                        in_=enc_t[hc * P : (hc + 1) * P, j0 : j0 + NI // 2],
                    )

            # ---- stage 1: v_psum[p, jc] = v[jc*128+p] ----
            # jc-outer: each v column's accumulation group must fully close
            # (stop=True) before the next column's start=True, because all 8
            # columns share one 2 KiB PSUM zero region and start re-marks the
            # whole region pending-zero.
            v_psum = psp.tile([P, HC], f32)
            for jc in range(HC):
                for kc in range(HC):
                    wi, kk = divmod(kc, 4)
                    lhsT = w_sb[wi][:, kk, jc * P : (jc + 1) * P]
                    nc.tensor.matmul(
                        v_psum[:, jc : jc + 1],
                        lhsT,
                        h_col[:, kc : kc + 1],
                        start=(kc == 0),
                        stop=(kc == HC - 1),
                    )
            v_col = const.tile([P, HC], bf16)
            nc.vector.tensor_copy(v_col[:], v_psum[:])

            # ---- stage 2: 16 groups of 512 rows; group g = c*4 + b lands at
            # PSUM partition 32c, bank b (columns b*512..) ----
            ps_out = psp.tile([P, 4 * 512], f32)
            for hc in range(HC):
                for half in range(2):
                    for c in (2 * half, 2 * half + 1):
                        for bk in range(4):
                            g = c * 4 + bk
                            nc.tensor.matmul(
                                ps_out[32 * c : 32 * c + 1, bk * 512 : (bk + 1) * 512],
                                v_col[:, hc : hc + 1],
                                e_sb[hc][:, g * 512 : (g + 1) * 512],
                                start=(hc == 0),
                                stop=(hc == HC - 1),
                                tile_position=(0, 32 * c),
                            )

            # ---- tail: bias add (PSUM->SBUF) + strided writeback ----
            out_sb = const.tile([P, 4 * 512], f32)
            nc.vector.tensor_scalar_add(out_sb[:], ps_out[:], bias_col[:])
            nc.sync.dma_start(out=out[:, :], in_=out_sb[0 : 3 * 32 + 1 : 32, :])
    nc.compile()
    return nc


def _get_nc():
    if "nc" not in _NC_CACHE:
        _NC_CACHE["nc"] = _build()
    return _NC_CACHE["nc"]


def kernel(hidden=None, encoder_hiddens=None, input_lengths=None, W=None, b=None):
    global LAST_RESULTS
    from concourse.bass_utils import run_bass_kernel_spmd

    hidden = np.asarray(hidden, dtype=np.float32)
    enc = np.asarray(encoder_hiddens, dtype=np.float32)
    W_ = np.asarray(W, dtype=np.float32)
    b_ = np.asarray(b, dtype=np.float32).reshape(1, 1)

    wt_bf = np.ascontiguousarray(W_.T.astype(BF))
    enc_bf = enc.astype(BF)  # [B, N, I, H]

    nc = _get_nc()
    in_maps = []
    for core in range(N_CORES):
        in_maps.append(
            {
                "enc_t": np.ascontiguousarray(enc_bf[core].reshape(NI, H).T),
                "wt": wt_bf,
                "h": np.ascontiguousarray(hidden[core].reshape(HC, P).T.astype(BF)),
                "bias": b_,
            }
        )
    res = run_bass_kernel_spmd(nc, in_maps, core_ids=list(range(N_CORES)))
    LAST_RESULTS = res
    # out[c, b*512 + r] = row (c*4+b)*512 + r of the flattened [8192] output
    out = np.stack(
        [res.results[i]["out"].reshape(NI).reshape(N, I) for i in range(N_CORES)]
    )
    return np.ascontiguousarray(out.astype(np.float32))
